# revision 21
# baseline (speedup 1.0000x reference)
"""Cross-Spatial-Attention Trainium2 kernel (8 NeuronCores, spatial sharding).

Strategy: shard the 256-row image into 8 bands of 32 rows (both batch elements
on every core, 1-row halos for the 3x3 depthwise convs). All convolutions and
the attention application are then fully local; the only cross-core data is the
channel-gram / norm / mean statistics (one small per-batch AllReduce).

Key formulations:
  - dwconv3x3(conv1x1(x)) == sum over 9 taps of shifted matmuls with
    per-tap-combined weights (PSUM accumulation) for the qk and v branches.
  - q,k are produced directly transposed ([n,c] layout) via
    out_chunk = x_chunk^T @ W_tap^T, so the channel gram needs no transpose
    pass and the spatial gate `sa` is a per-partition scalar.
  - the y-branch depthwise conv runs on the Vector engine as 9 shifted
    multiply-accumulates over a FLAT (no column halo) y layout so every AP is
    one contiguous 4B-aligned run (DVE 2x eligible); a one-element-right-
    shifted copy serves the odd-parity column taps, and the row-seam
    contamination this introduces is fixed by 6 tiny edge-repair ops.
  - v means come from x window sums (DVE reductions + 9 tiny matmuls), so the
    stats AllReduce needs nothing from the v convs: batch 0's AllReduce issues
    right after qk(0) and hides under qk(1); batch 1's hides under the v convs
    and batch-0 projection.
  - softmax over a full 128x128 gram with a block-diagonal mask; the
    attention apply + output projection collapse into one matmul
    (Meff = proj @ attnBD), and the spectral gate folds into the
    projection weights for the dwconv(y) branch.
"""

import numpy as np
from contextlib import ExitStack

import concourse.bass as bass
import concourse.bacc as bacc
import concourse.tile as tile
from concourse import mybir
from concourse.bass_utils import run_bass_kernel_spmd

FP32 = mybir.dt.float32
BF16 = mybir.dt.bfloat16
AF = mybir.ActivationFunctionType
ALU = mybir.AluOpType

B, C, H, W = 2, 128, 256, 256
HD, DH = 8, 16
NCORES = 8
RPC = H // NCORES            # 32 rows per core
HH, WW = RPC + 2, W + 2      # 34 x 258 halo'd band (x only)
FREE = HH * WW               # 8772
NLOC = RPC * W               # 8192 output positions per band per batch
NCH_T = NLOC // 128          # 64 transposed chunks
NCH_A = NLOC // 512          # 16 layout-A chunks
NTOT = float(H * W)          # global spatial size
YF = HH * W                  # 8704: flat y tile (34 rows x 256 cols)
YROWS = 4                    # out rows per vector-engine dw chunk
NCH_Y = RPC // YROWS         # 8 chunks


def _emit(tc, io):
    nc = tc.nc
    ctx = ExitStack()

    wpool = ctx.enter_context(tc.tile_pool(name="wpool", bufs=1))
    xpool = ctx.enter_context(tc.tile_pool(name="xpool", bufs=2))
    ypool = ctx.enter_context(tc.tile_pool(name="ypool", bufs=2))
    yspool = ctx.enter_context(tc.tile_pool(name="yspool", bufs=2))
    vpool = ctx.enter_context(tc.tile_pool(name="vpool", bufs=2))
    y2pool = ctx.enter_context(tc.tile_pool(name="y2pool", bufs=2))
    spool = ctx.enter_context(tc.tile_pool(name="spool", bufs=1))
    rpool = ctx.enter_context(tc.tile_pool(name="rpool", bufs=6))
    mpool = ctx.enter_context(tc.tile_pool(name="mpool", bufs=2))
    opool = ctx.enter_context(tc.tile_pool(name="opool", bufs=3))
    psA = ctx.enter_context(tc.tile_pool(name="psA", bufs=2, space="PSUM"))
    psQK = ctx.enter_context(tc.tile_pool(name="psQK", bufs=3, space="PSUM"))
    psG = ctx.enter_context(tc.tile_pool(name="psG", bufs=1, space="PSUM"))
    dpool = ctx.enter_context(tc.tile_pool(name="dram", bufs=1, space="DRAM"))

    def wload(name, shape, dt=BF16, eng=None):
        t = wpool.tile(shape, dt, tag=name)
        (eng or nc.sync).dma_start(out=t[:], in_=io[name][:])
        return t

    # ---- small weights first (sync ring) ----
    saw1t = wload("saw1t", [128, 32])
    w2rep = wload("w2rep", [128, 32])
    w3rep = wload("w3rep", [128, 1])
    spw1t = wload("spw1t", [128, 16], FP32)
    spw2t = wload("spw2t", [16, 16], FP32)
    spw3t = wload("spw3t", [16, 128], FP32)
    projt = wload("projt", [128, 128], FP32)
    wydc = wload("wydc", [128, 18], FP32)  # cols 0:9 = dw taps, 9:18 = negated
    consts = wload("consts", [128, 386], FP32)
    eye = consts[:, 0:128]
    bdmask = consts[:, 128:256]
    tempp = consts[:, 256:257]
    onesrow = consts[0:1, 257:385]

    # ---- bulk inputs spread over the 3 dynamic rings; critical tiles first ----
    xts, yfs, yss, vts, y2ts, saTs = [], [], [], [], [], []
    HALFX = FREE // 2
    HALFY = YF // 2
    yf0 = ypool.tile([128, YF], BF16, tag="yf", name="yf0")
    yd0 = io["yf"][0].rearrange("c h w -> c (h w)")
    nc.gpsimd.dma_start(out=yf0[:, 0:HALFY], in_=yd0[:, 0:HALFY])
    nc.scalar.dma_start(out=yf0[:, HALFY:YF], in_=yd0[:, HALFY:YF])
    yfs.append(yf0)
    xt0 = xpool.tile([128, FREE], BF16, tag="x", name="xt0")
    xd0 = io["xh"][0].rearrange("c h w -> c (h w)")
    nc.sync.dma_start(out=xt0[:, 0:HALFX], in_=xd0[:, 0:HALFX])
    nc.gpsimd.dma_start(out=xt0[:, HALFX:FREE], in_=xd0[:, HALFX:FREE])
    xts.append(xt0)
    w9qk = wload("w9qk", [128, 9 * 256], eng=nc.scalar)
    ys0 = yspool.tile([128, YF + 2], BF16, tag="ys", name="ys0")
    nc.sync.dma_start(out=ys0[:], in_=io["ys"][0])
    yss.append(ys0)
    yf1 = ypool.tile([128, YF], BF16, tag="yf", name="yf1")
    nc.gpsimd.dma_start(out=yf1[:], in_=io["yf"][1].rearrange("c h w -> c (h w)"))
    yfs.append(yf1)
    xt1 = xpool.tile([128, FREE], BF16, tag="x", name="xt1")
    nc.scalar.dma_start(out=xt1[:], in_=io["xh"][1].rearrange("c h w -> c (h w)"))
    xts.append(xt1)
    w9v = wload("w9v", [128, 9 * 128], eng=nc.sync)
    ys1 = yspool.tile([128, YF + 2], BF16, tag="ys", name="ys1")
    nc.gpsimd.dma_start(out=ys1[:], in_=io["ys"][1])
    yss.append(ys1)

    # ---- y depthwise conv steps (vector engine over flat layout) ----
    # yf = y row-major (34 x 256, vertical halo rows included); ys[m] = yf[m-1]
    # (one-element right shift, ys[0] = left-pad 0).  For out rows r0..r0+3:
    #   tj=1 reads yf at (r0+ti)*256      (exact)
    #   tj=0 reads ys at (r0+ti)*256      (= yf[.-1]; row-seam dirt at w=0)
    #   tj=2 reads ys at (r0+ti)*256 + 2  (= yf[.+1]; row-seam dirt at w=255)
    ydw_early, ydw_late = [], []

    def make_ydw(b):
        yf, ys = yfs[b], yss[b]
        y2t = y2pool.tile([128, NLOC], BF16, tag="y2")
        y2ts.append(y2t)
        for ch in range(NCH_Y):
            r0 = YROWS * ch
            ov = y2t[:, 1024 * ch:1024 * ch + 1024]
            for ii, t in enumerate((1, 4, 7)):      # exact taps (init first)
                ti = t // 3
                iv = yf[:, (r0 + ti) * 256:(r0 + ti) * 256 + 1024]
                wc = wydc[:, t:t + 1]
                if ii == 0:
                    ydw_early.append(
                        lambda ov=ov, iv=iv, wc=wc:
                        nc.vector.tensor_scalar_mul(ov, iv, wc))
                else:
                    ydw_early.append(
                        lambda ov=ov, iv=iv, wc=wc:
                        nc.vector.scalar_tensor_tensor(
                            ov, iv, wc, ov, ALU.mult, ALU.add))
            for t in (0, 2, 3, 5, 6, 8):            # shifted taps
                ti, tj = t // 3, t % 3
                off = (r0 + ti) * 256 + (2 if tj == 2 else 0)
                iv = ys[:, off:off + 1024]
                wc = wydc[:, t:t + 1]
                ydw_late.append(
                    lambda ov=ov, iv=iv, wc=wc:
                    nc.vector.scalar_tensor_tensor(
                        ov, iv, wc, ov, ALU.mult, ALU.add))
        # row-seam repairs: subtract the wrongly-read neighbor-row element
        y23 = y2t[:].rearrange("p (r w) -> p r w", w=256)
        yf3 = yf[:].rearrange("p (r w) -> p r w", w=256)
        for t in (0, 3, 6):                          # tj=0 dirt at w=0
            ti = t // 3
            rlo = 1 if ti == 0 else 0
            ov = y23[:, rlo:RPC, 0:1]
            iv = yf3[:, rlo + ti - 1:RPC + ti - 1, 255:256]
            ydw_late.append(
                lambda ov=ov, iv=iv, wc=wydc[:, 9 + t:10 + t]:
                nc.vector.scalar_tensor_tensor(
                    ov, iv, wc, ov, ALU.mult, ALU.add))
        for t in (2, 5, 8):                          # tj=2 dirt at w=255
            ti = t // 3
            rhi = RPC - 1 if ti == 2 else RPC
            ov = y23[:, 0:rhi, 255:256]
            iv = yf3[:, ti + 1:rhi + ti + 1, 0:1]
            ydw_late.append(
                lambda ov=ov, iv=iv, wc=wydc[:, 9 + t:10 + t]:
                nc.vector.scalar_tensor_tensor(
                    ov, iv, wc, ov, ALU.mult, ALU.add))

    def pop_ydw(n):
        for _ in range(n):
            if ydw_early:
                ydw_early.pop(0)()
            elif ydw_late:
                ydw_late.pop(0)()

    def drain_ydw():
        pop_ydw(len(ydw_early) + len(ydw_late))

    arst = [mpool.tile([128, 131], FP32, tag="arst0", name="arst0"),
            mpool.tile([128, 131], FP32, tag="arst1", name="arst1")]
    arres = [mpool.tile([128, 131], FP32, tag="arres0", name="arres0"),
             mpool.tile([128, 131], FP32, tag="arres1", name="arres1")]

    # ================= per-batch: sa gate + qk conv/gram + AllReduce ========
    for b in range(B):
        xt, yf = xts[b], yfs[b]
        make_ydw(b)

        # ---- spatial-attention gate: sa = sigmoid(w3 relu(w2 relu(w1 y))) ----
        s1 = spool.tile([128, 2048], BF16, tag="s1")
        s2 = spool.tile([128, 2048], BF16, tag="s2")
        for g in range(4):
            ps1 = psA.tile([128, 512], FP32, tag="a")
            for k in range(4):
                nn = 4 * g + k
                yv = yf[:, (2 * nn + 1) * 256:(2 * nn + 1) * 256 + 512]
                nc.tensor.matmul(ps1[32 * k:32 * k + 32, :], saw1t[:, :], yv,
                                 start=True, stop=True, tile_position=(0, 32 * k))
            if g % 2 == 0:
                nc.vector.tensor_scalar_max(s1[:, 512 * g:512 * g + 512], ps1[:, :], 0.0)
            else:
                nc.scalar.activation(s1[:, 512 * g:512 * g + 512], ps1[:, :], AF.Relu)
        for g in range(4):
            ps2 = psA.tile([128, 512], FP32, tag="a")
            for k in range(4):
                nc.tensor.matmul(ps2[32 * k:32 * k + 32, :],
                                 w2rep[32 * k:32 * k + 16, :],
                                 s1[32 * k:32 * k + 16, 512 * g:512 * g + 512],
                                 start=True, stop=True,
                                 tile_position=(32 * k, 32 * k))
            if g % 2 == 0:
                nc.vector.tensor_scalar_max(s2[:, 512 * g:512 * g + 512], ps2[:, :], 0.0)
            else:
                nc.scalar.activation(s2[:, 512 * g:512 * g + 512], ps2[:, :], AF.Relu)
        # stage 3: saT[n] packed as [128, 64] (col j holds n = 128j + p)
        sv_ps = psG.tile([128, 66], FP32, tag="sv")
        for j in range(NCH_T):
            nn, off = j // 4, (j % 4) * 128
            g, k = nn // 4, nn % 4
            nc.tensor.matmul(sv_ps[:, j:j + 1],
                             s2[32 * k:32 * k + 16,
                                512 * g + off:512 * g + off + 128],
                             w3rep[32 * k:32 * k + 16, :],
                             start=True, stop=True, tile_position=(32 * k, 0))
        saT = mpool.tile([128, 64], FP32, tag="saT")
        nc.scalar.activation(saT[:], sv_ps[:, 0:64], AF.Sigmoid)
        saTs.append(saT)

        # ---- v-mean from x window sums (S_t = shifted 32x256 window sums) ----
        x3 = xt[:].rearrange("p (h w) -> p h w", h=HH)
        rsum = mpool.tile([128, HH], FP32, tag="rsum")
        nc.vector.tensor_reduce(rsum[:], x3[:, :, 0:256], mybir.AxisListType.X,
                                ALU.add)
        bsum = mpool.tile([128, 3], FP32, tag="bsum")
        for ti in range(3):
            nc.vector.tensor_reduce(bsum[:, ti:ti + 1], rsum[:, ti:ti + 32],
                                    mybir.AxisListType.X, ALU.add)
        x3w = xt[:].rearrange("p (h w) -> p w h", h=HH)
        csum = mpool.tile([128, 12], FP32, tag="csum")
        for wi, w in enumerate((0, 1, 256, 257)):
            for ti in range(3):
                nc.vector.tensor_reduce(csum[:, 3 * wi + ti:3 * wi + ti + 1],
                                        x3w[:, w:w + 1, ti:ti + 32],
                                        mybir.AxisListType.X, ALU.add)
        stap = mpool.tile([128, 9], FP32, tag="stap")
        for ti in range(3):
            nc.vector.tensor_copy(stap[:, 3 * ti:3 * ti + 1], bsum[:, ti:ti + 1])
            nc.vector.tensor_tensor(stap[:, 3 * ti + 1:3 * ti + 2],
                                    stap[:, 3 * ti:3 * ti + 1],
                                    csum[:, ti:ti + 1], ALU.subtract)
            nc.vector.tensor_tensor(stap[:, 3 * ti + 1:3 * ti + 2],
                                    stap[:, 3 * ti + 1:3 * ti + 2],
                                    csum[:, 6 + ti:7 + ti], ALU.add)
            nc.vector.tensor_tensor(stap[:, 3 * ti + 2:3 * ti + 3],
                                    stap[:, 3 * ti + 1:3 * ti + 2],
                                    csum[:, 3 + ti:4 + ti], ALU.subtract)
            nc.vector.tensor_tensor(stap[:, 3 * ti + 2:3 * ti + 3],
                                    stap[:, 3 * ti + 2:3 * ti + 3],
                                    csum[:, 9 + ti:10 + ti], ALU.add)
        stapb = mpool.tile([128, 9], BF16, tag="stapb")
        nc.vector.tensor_copy(stapb[:], stap[:])
        for t in range(9):
            nc.tensor.matmul(sv_ps[:, 64:65], w9v[:, 128 * t:128 * t + 128],
                             stapb[:, t:t + 1], start=(t == 0), stop=(t == 8))
        nc.vector.tensor_copy(arst[b][:, 130:131], sv_ps[:, 64:65])

        # ---- qk conv (transposed layout) + gram accumulation ----
        Gt = psG.tile([128, 384], FP32, tag="G")
        for j in range(NCH_T):
            r, c0 = j // 2, (j % 2) * 128
            pqk = psQK.tile([128, 256], FP32, tag="qk")
            for t in range(9):
                ti, tj = t // 3, t % 3
                base = (r + ti) * WW + c0 + tj
                nc.tensor.matmul(pqk[:, :], xt[:, base:base + 128],
                                 w9qk[:, 256 * t:256 * t + 256],
                                 start=(t == 0), stop=(t == 8))
            rt = rpool.tile([128, 256], BF16, tag="ring")
            # q scaled by sa (per-partition in transposed layout), k plain
            nc.scalar.activation(rt[:, 0:128], pqk[:, 0:128], AF.Copy,
                                 scale=saT[:, j:j + 1])
            nc.vector.tensor_copy(rt[:, 128:256], pqk[:, 128:256])
            nc.tensor.matmul(Gt[:, 0:256], rt[:, 0:128], rt[:, 0:256],
                             start=(j == 0), stop=(j == NCH_T - 1),
                             skip_group_check=True)
            nc.tensor.matmul(Gt[:, 256:384], rt[:, 128:256], rt[:, 128:256],
                             start=(j == 0), stop=(j == NCH_T - 1),
                             skip_group_check=True)
            pop_ydw(2)

        # ---- stats staging + this batch's AllReduce ----
        junk = mpool.tile([128, 128], FP32, tag="junk")
        nc.vector.tensor_copy(arst[b][:, 0:128], Gt[:, 128:256])  # Gqk
        nc.vector.scalar_tensor_tensor(junk[:], Gt[:, 0:128], 1.0, eye,
                                       ALU.mult, ALU.mult,
                                       accum_out=arst[b][:, 128:129])
        nc.vector.scalar_tensor_tensor(junk[:], Gt[:, 256:384], 1.0, eye,
                                       ALU.mult, ALU.mult,
                                       accum_out=arst[b][:, 129:130])
        din = dpool.tile([128, 131], FP32, tag=f"din{b}", name=f"din{b}")
        dout = dpool.tile([128, 131], FP32, tag=f"dout{b}", name=f"dout{b}")
        nc.sync.dma_start(out=din[:], in_=arst[b][:])
        nc.gpsimd.collective_compute(
            "AllReduce", ALU.add,
            replica_groups=[list(range(NCORES))],
            ins=[din[:].opt()], outs=[dout[:].opt()])
        nc.sync.dma_start(out=arres[b][:], in_=dout[:])

    # ================= post-AllReduce math / v convs / projection ==========
    meffts, attnts, p2ts = [], [], []

    def post_ar(b):
        # 1/max(sqrt(d), eps) with one Newton-rsqrt refinement
        rqk = mpool.tile([128, 2], FP32, tag="rqk")
        srt = mpool.tile([128, 2], FP32, tag="srt")
        dcat = arres[b][:, 128:130]  # [qd kd]
        nc.scalar.activation(srt[:], dcat, AF.Sqrt)
        nc.vector.tensor_scalar_max(srt[:], srt[:], 1e-12)
        nc.vector.reciprocal(rqk[:], srt[:])
        r2 = mpool.tile([128, 2], FP32, tag="r2")
        nc.vector.tensor_tensor(r2[:], rqk[:], rqk[:], ALU.mult)
        nc.vector.tensor_tensor(r2[:], r2[:], dcat, ALU.mult)
        nc.vector.tensor_scalar(r2[:], r2[:], -0.5, 1.5, ALU.mult, ALU.add)
        nc.vector.tensor_tensor(rqk[:], rqk[:], r2[:], ALU.mult)
        rqt = mpool.tile([128, 1], FP32, tag="rqt")
        nc.vector.tensor_tensor(rqt[:], rqk[:, 0:1], tempp, ALU.mult)

        # broadcast rk along partitions: rkb[p, d] = rk[d]
        ps1 = psA.tile([128, 128], FP32, tag="a")
        nc.tensor.matmul(ps1[0:1, :], rqk[:, 1:2], eye, start=True, stop=True)
        rkrow = mpool.tile([1, 128], FP32, tag="rkrow")
        nc.scalar.copy(rkrow[:], ps1[0:1, :])
        ps2 = psA.tile([128, 128], FP32, tag="a")
        nc.tensor.matmul(ps2[:, :], onesrow, rkrow[:], start=True, stop=True)

        # logits -> masked softmax -> attnBD
        gh = mpool.tile([128, 128], FP32, tag="gh")
        nc.vector.scalar_tensor_tensor(gh[:], arres[b][:, 0:128], rqt[:, 0:1],
                                       ps2[:, :], ALU.mult, ALU.mult)
        sm = mpool.tile([128, 128], FP32, tag="sm")
        nc.scalar.activation(sm[:], gh[:], AF.Exp)
        rs = mpool.tile([128, 1], FP32, tag="rs")
        nc.vector.scalar_tensor_tensor(sm[:], sm[:], 1.0, bdmask,
                                       ALU.mult, ALU.mult, accum_out=rs[:])
        nc.vector.reciprocal(rs[:], rs[:])
        attn = mpool.tile([128, 128], FP32, tag="attn")
        nc.vector.tensor_scalar_mul(attn[:], sm[:], rs[:, 0:1])

        # MeffT = (proj @ attnBD)^T: lhsT=attn, rhs=projT
        psM = psA.tile([128, 128], FP32, tag="a")
        nc.tensor.matmul(psM[:, :], attn[:], projt[:], start=True, stop=True)
        mefft = mpool.tile([128, 128], BF16, tag="mefft")
        nc.scalar.copy(mefft[:], psM[:, :])
        meffts.append(mefft)

        # attn^T for pooled = attnBD @ v_mean
        psT = psA.tile([128, 128], FP32, tag="a")
        nc.tensor.transpose(psT[:, :], attn[:], eye)
        attnt = mpool.tile([128, 128], FP32, tag="attnt")
        nc.vector.tensor_copy(attnt[:], psT[:, :])
        attnts.append(attnt)

        # spectral gate MLP on pooled
        psP = psA.tile([128, 1], FP32, tag="a")
        nc.tensor.matmul(psP[:, :], attnt[:], arres[b][:, 130:131],
                         start=True, stop=True)
        pooled = mpool.tile([128, 1], FP32, tag="pooled")
        nc.scalar.activation(pooled[:], psP[:, :], AF.Copy, scale=1.0 / NTOT)
        psg1 = psA.tile([16, 1], FP32, tag="a")
        nc.tensor.matmul(psg1[:, :], spw1t[:], pooled[:], start=True, stop=True)
        g1 = mpool.tile([16, 1], FP32, tag="g1")
        nc.scalar.activation(g1[:], psg1[:, :], AF.Gelu)
        psg2 = psA.tile([16, 1], FP32, tag="a")
        nc.tensor.matmul(psg2[:, :], spw2t[:], g1[:], start=True, stop=True)
        g2 = mpool.tile([16, 1], FP32, tag="g2")
        nc.scalar.activation(g2[:], psg2[:, :], AF.Gelu)
        psg3 = psA.tile([128, 1], FP32, tag="a")
        nc.tensor.matmul(psg3[:, :], spw3t[:], g2[:], start=True, stop=True)
        spec = mpool.tile([128, 1], FP32, tag="spec")
        nc.scalar.activation(spec[:], psg3[:, :], AF.Sigmoid)
        p2t = mpool.tile([128, 128], BF16, tag="p2t")
        nc.vector.tensor_scalar_mul(p2t[:], projt[:], spec[:, 0:1])
        p2ts.append(p2t)

    def v_conv(b):
        xt = xts[b]
        vt = vpool.tile([128, NLOC], BF16, tag="v")
        vts.append(vt)
        for nn in range(NCH_A):
            r0 = 2 * nn
            pv = psA.tile([128, 512], FP32, tag="a")
            for t in range(9):
                ti, tj = t // 3, t % 3
                xv = xt[:].rearrange("p (h w) -> p h w", h=HH)[
                    :, r0 + ti:r0 + ti + 2, tj:tj + 256]
                nc.tensor.matmul(pv[:, :], w9v[:, 128 * t:128 * t + 128], xv,
                                 start=(t == 0), stop=(t == 8))
            nc.scalar.copy(vt[:, 512 * nn:512 * nn + 512], pv[:, :])
            pop_ydw(1)

    def proj(b):
        out2d = io["out"][b].rearrange("c h w -> c (h w)")
        for nn in range(NCH_A):
            pf = psA.tile([128, 512], FP32, tag="a")
            nc.tensor.matmul(pf[:, :], meffts[b][:],
                             vts[b][:, 512 * nn:512 * nn + 512],
                             start=True, stop=False)
            nc.tensor.matmul(pf[:, :], p2ts[b][:],
                             y2ts[b][:, 512 * nn:512 * nn + 512],
                             start=False, stop=True)
            ot = opool.tile([128, 512], FP32, tag="ot")
            if nn % 2 == 0:
                nc.scalar.copy(ot[:], pf[:, :])
                nc.sync.dma_start(out=out2d[:, 512 * nn:512 * nn + 512], in_=ot[:])
            else:
                nc.vector.tensor_copy(ot[:], pf[:, :])
                nc.gpsimd.dma_start(out=out2d[:, 512 * nn:512 * nn + 512], in_=ot[:])

    post_ar(0)
    v_conv(0)
    drain_ydw()
    proj(0)
    v_conv(1)
    post_ar(1)
    proj(1)

    ctx.close()


def build_nc():
    nc = bacc.Bacc("TRN2", target_bir_lowering=False, debug=False,
                   num_devices=NCORES)
    io = {}

    def inp(name, shape, dt):
        io[name] = nc.dram_tensor(name, shape, dt, kind="ExternalInput")

    inp("xh", [B, C, HH, WW], BF16)
    inp("yf", [B, C, HH, W], BF16)
    inp("ys", [B, C, HH * W + 2], BF16)
    inp("w9qk", [128, 9 * 256], BF16)
    inp("w9v", [128, 9 * 128], BF16)
    inp("saw1t", [128, 32], BF16)
    inp("w2rep", [128, 32], BF16)
    inp("w3rep", [128, 1], BF16)
    inp("spw1t", [128, 16], FP32)
    inp("spw2t", [16, 16], FP32)
    inp("spw3t", [16, 128], FP32)
    inp("projt", [128, 128], FP32)
    inp("wydc", [128, 18], FP32)
    inp("consts", [128, 386], FP32)
    io["out"] = nc.dram_tensor("out", [B, C, RPC, W], FP32, kind="ExternalOutput")

    with tile.TileContext(nc) as tc:
        _emit(tc, io)
    nc.finalize()
    return nc


_CACHE = {}


def _prep_host(x, y, qkv_w, qkv_dw_w, proj_w, sa_w1, sa_w2, sa_w3,
               sp_w1, sp_w2, sp_w3, dw_w, temperature):
    import ml_dtypes
    bf = ml_dtypes.bfloat16
    f32 = np.float32

    x = np.asarray(x, f32)
    y = np.asarray(y, f32)
    xp = np.zeros((B, C, H + 2, W + 2), f32)
    xp[:, :, 1:H + 1, 1:W + 1] = x
    yv = np.zeros((B, C, H + 2, W), f32)      # vertical halo only, flat cols
    yv[:, :, 1:H + 1, :] = y
    xp = xp.astype(bf)
    yv = yv.astype(bf)

    qkv_w = np.asarray(qkv_w, f32)
    dw = np.asarray(qkv_dw_w, f32).reshape(3 * C, 9)
    w9qk = np.concatenate(
        [(qkv_w[:256] * dw[:256, t:t + 1]).T for t in range(9)], axis=1)
    w9v = np.concatenate(
        [(qkv_w[256:] * dw[256:, t:t + 1]).T for t in range(9)], axis=1)
    wydc = np.zeros((C, 18), f32)
    wydc[:, 0:9] = np.asarray(dw_w, f32).reshape(C, 9)
    wydc[:, 9:18] = -wydc[:, 0:9]

    w2rep = np.zeros((128, 32), f32)
    w3rep = np.zeros((128, 1), f32)
    for k in range(4):
        w2rep[32 * k:32 * k + 16, 0:16] = np.asarray(sa_w2, f32).T
        w3rep[32 * k:32 * k + 16] = np.asarray(sa_w3, f32).T
    saw1tp = np.zeros((128, 32), f32)
    saw1tp[:, 0:16] = np.asarray(sa_w1, f32).T

    consts = np.zeros((128, 386), f32)
    consts[:, 0:128] = np.eye(128, dtype=f32)
    ci = np.arange(128) // DH
    consts[:, 128:256] = (ci[:, None] == ci[None, :]).astype(f32)
    consts[:, 256] = np.asarray(temperature, f32).reshape(HD)[ci]
    consts[0, 257:385] = 1.0

    common = {
        "w9qk": w9qk.astype(bf), "w9v": w9v.astype(bf),
        "saw1t": saw1tp.astype(bf),
        "w2rep": w2rep.astype(bf), "w3rep": w3rep.astype(bf),
        "spw1t": np.asarray(sp_w1, f32).T.copy(),
        "spw2t": np.asarray(sp_w2, f32).T.copy(),
        "spw3t": np.asarray(sp_w3, f32).T.copy(),
        "projt": np.asarray(proj_w, f32).T.copy(),
        "wydc": wydc,
        "consts": consts,
    }
    in_maps = []
    for i in range(NCORES):
        m = dict(common)
        m["xh"] = np.ascontiguousarray(xp[:, :, 32 * i:32 * i + HH, :])
        yb = np.ascontiguousarray(yv[:, :, 32 * i:32 * i + HH, :])
        m["yf"] = yb
        flat = yb.reshape(B, C, HH * W)
        ys = np.zeros((B, C, HH * W + 2), yb.dtype)
        ys[:, :, 1:HH * W + 1] = flat              # ys[m] = yf[m-1]
        m["ys"] = ys
        in_maps.append(m)
    return in_maps


def kernel(**inputs):
    if "nc" not in _CACHE:
        _CACHE["nc"] = build_nc()
    nc = _CACHE["nc"]
    in_maps = _prep_host(**inputs)
    res = run_bass_kernel_spmd(nc, in_maps, core_ids=list(range(NCORES)))
    shards = [res.results[i]["out"] for i in range(NCORES)]
    return np.concatenate(shards, axis=2).astype(np.float32)
